# revision 40
# baseline (speedup 1.0000x reference)
"""Trainium2 Bass kernel for CoreSageLayer (GNN mean-aggregate + 3-way linear).

Computation (reference):
    mask = (adj == 1)                      # [N, N] 0/1
    deg  = mask.sum(axis=1)                # [N]
    x1   = (mask @ x) / deg[:, None]       # [N, F]
    out[k] = concat([x1, x], 1) @ W[k] + bias   # [3, N, O]

Distribution: row-shard adj / x1 / out over nodes across 8 cores; replicate
x and weights; no collectives (rows independent).

Device schedule per core (NB=1024 nodes), fp8 DoubleRow stage 1:
  Nodes are processed in two sweeps of 512 (one PSUM bank of output each).
  Per sweep, the 8192-deep contraction streams as 32 mask half-slabs
  [128, 2, 512] (fp8, 0/1 exact); stationary = x pair-chunks [128, 2, 128]
  (fp8) giving msumT = (mask @ x)^T accumulated directly in [feature, node]
  orientation (no transposes), plus one ones-stationary matmul per slab for
  deg. Stage 2 for sweep-0 nodes is interleaved into sweep 1's PE stream;
  epilogue out[k] = (msumT_j @ W1k)*rec + xT_j @ W2k splits across the ACT
  engine (per-partition rec scale out of PSUM) and the DVE (add + bf16 cast).
  Output is written bf16 node-major [NB, 3, O] (one DMA per node tile) and
  upcast/transposed on host.
"""

import sys

sys.path.insert(0, "/opt/trn_rl_repo")

import numpy as np

N = 8192
F = 256
O = 256
NCORES = 8
NB = N // NCORES          # nodes per core (1024)
PC = N // 256             # contraction pair-chunks of 256 rows (32)
JT = NB // 128            # stage-2 node tiles per core (8)
NBLK = 512                # nodes per sweep (one PSUM bank)
NSW = NB // NBLK          # sweeps (2)
JSW = NBLK // 128         # stage-2 node tiles per sweep (4)


def _patch_tile_drain():
    """This container's walrus allows only one sync-wait per CTRL instruction;
    split the Tile kernel-tail drain's waits onto single-wait no-fuse NoOps."""
    import concourse.tile as tile
    from concourse import mybir
    from concourse.tile import ScopedClock

    if getattr(tile.TileContext, "_drain_split_patched", False):
        return

    def _drain_and_barrier(self, tick_clock, wait_clock):
        nc = self.nc
        drain_inst = nc.sync.drain()
        wait_clock.add_sem_waits(
            drain_inst.ins, ScopedClock({None: tick_clock.global_clock})
        )
        si = drain_inst.ins.sync_info
        if si is not None and len(si.on_wait) > 1:
            waits = list(si.on_wait)
            drain_inst.ins.sync_info = mybir.SyncInfo(
                on_wait=[waits[0]], on_update=list(si.on_update)
            )
            for w in waits[1:]:
                nop = nc.sync.nop(nofuse=True, hint="split_wait")
                nop.ins.sync_info = mybir.SyncInfo(on_wait=[w], on_update=[])
        nc.all_engine_barrier()
        assert self.sems is not None
        popped = nc._tile_sem_poison_stack.pop()
        assert popped is self._sem_poison
        nc.clear_and_free_semaphores(list(self.sems.allocated().values()))
        nc.all_engine_barrier()

    tile.TileContext._drain_and_barrier = _drain_and_barrier
    tile.TileContext._drain_split_patched = True

    # Same walrus limitation, general case: any instruction that Tile gave
    # >1 sem-wait (e.g. a DMA with both RAW and WAR deps) fails codegen.
    # Split surplus waits onto fresh single-wait NoOps emitted just before
    # the instruction on the same engine, at the serialized-BIR level.
    import concourse.bass as bass
    import orjson

    _orig_to_json_bytes = bass.Bass.to_json_bytes

    def _to_json_bytes_split(self):
        m = orjson.loads(_orig_to_json_bytes(self))
        ctr = 0
        for fn in m.get("functions", []):
            for bb in fn.get("blocks", []):
                insts = bb.get("instructions", [])
                new = []
                for inst in insts:
                    si = inst.get("sync_info")
                    waits = (si or {}).get("on_wait") or []
                    if len(waits) > 1:
                        for w in waits[:-1]:
                            ctr += 1
                            new.append({
                                "name": f"SWNOP-{ctr}",
                                "opcode": "NoOp",
                                "engine": inst["engine"],
                                "ins": [],
                                "outs": [],
                                "sync_info": {"on_wait": [w], "on_update": []},
                            })
                        si["on_wait"] = [waits[-1]]
                    new.append(inst)
                bb["instructions"] = new
        return orjson.dumps(m)

    bass.Bass.to_json_bytes = _to_json_bytes_split


def build_bass(with_bias: bool):
    import concourse.bass as bass
    import concourse.tile as tile
    from concourse import mybir
    from concourse.masks import make_identity

    _patch_tile_drain()

    f8 = mybir.dt.float8e4
    f32 = mybir.dt.float32
    bf16 = mybir.dt.bfloat16
    DR = mybir.MatmulPerfMode.DoubleRow
    Copy = mybir.ActivationFunctionType.Copy

    GRP = 8  # pair-chunks per mask DMA (1MB transfers for full DMA-engine BW)

    nc = bass.Bass()
    # mask half-slabs, grouped for DMA:
    # [sb, g, p, u, i, n] = mask[node sb*512+n, m=(g*8+u)*256+i*128+p]
    mq = nc.dram_tensor("mq", [NSW, PC // GRP, 128, GRP, 2, NBLK], f8,
                        kind="ExternalInput")
    # x stationary pair-chunks: [p, pc*2+h, i, f] = x[pc*256+i*128+p, h*128+f]
    xq = nc.dram_tensor("xq", [128, PC * 2, 2, 128], f8, kind="ExternalInput")
    # stage-2 x^T: [p, fc, n] = x[core node n, fc*128+p]
    xt = nc.dram_tensor("xt", [128, 2, NB], bf16, kind="ExternalInput")
    # weights: [p, k, fc, o] = W[k, fc*128+p, o]
    w = nc.dram_tensor("w", [128, 3, 4, O], bf16, kind="ExternalInput")
    # x1-block weights as fp8 pairs for the stage-2 DoubleRow matmul:
    # [p, k, i, o] = W[k, i*128+p, o]
    w8 = nc.dram_tensor("w8", [128, 3, 2, O], f8, kind="ExternalInput")
    if with_bias:
        biasr = nc.dram_tensor("biasr", [128, O], f32, kind="ExternalInput")
    # node-major output: [n, k, o] (host transposes back to [3, NB, O])
    out = nc.dram_tensor("out", [NB, 3, O], bf16, kind="ExternalOutput")

    with tile.TileContext(nc) as tc:
        # two pools only: each pool exit costs a multi-barrier cleanup round
        # in the kernel tail (~1µs apiece)
        with (
            tc.tile_pool(name="sb", bufs=1) as const_pool,
            tc.tile_pool(name="ps", bufs=1, space="PSUM") as ps_pool,
        ):
            mask_pool = work_pool = const_pool
            psx1_pool = psdeg_pool = ps2_pool = ps_pool
            xt_sb = const_pool.tile([128, 2, NB], bf16)
            w_sb = const_pool.tile([128, 3, 4, O], bf16)
            w8_sb = const_pool.tile([128, 3, 2, O], f8)
            if with_bias:
                bias_sb = const_pool.tile([128, O], f32)

            xq_sb = const_pool.tile([128, PC * 2, 2, 128], f8)
            ones_sb = const_pool.tile([128, 2, 16], f8)
            nc.vector.memset(ones_sb[:], 0.0)
            nc.vector.memset(ones_sb[:, :, 0:1], 1.0)
            # warm the ACT table during the initial DMA wait: the first
            # ACTIVATE otherwise pays a 1.3µs ACT_TABLE_LOAD mid-kernel
            actwarm = const_pool.tile([128, 1], f32)
            nc.vector.memset(actwarm[:], 1.0)
            nc.scalar.activation(actwarm[:], actwarm[:], Copy, scale=1.0)
            identity = const_pool.tile([128, 128], f32)
            make_identity(nc, identity)
            # msumT as fp8 feature-pairs: [p, i, n] = msumT[i*128+p, n]
            x1t8_sb = const_pool.tile([128, 2, NB], f8)
            deg_sb = const_pool.tile([16, NB], f32)
            rec_sb = const_pool.tile([128, 16 * JT], f32)

            # xq pieces ride the same sync queue as the mask groups, in
            # consumption order (per-queue FIFO = DMA priority; a second
            # queue would steal engines from the critical path)
            def xq_piece(lo, hi):
                nc.sync.dma_start(xq_sb[:, 2 * lo:2 * hi],
                                  xq[:, 2 * lo:2 * hi])

            x1ps_s = {}
            degps_s = {}
            psumt_s = {}

            def emit_sweep(sb, interleave=None, after_group_dma=None):
                x1ps = [psx1_pool.tile([128, NBLK], f32, tag=f"x1_{h}",
                                       name=f"x1ps{sb}{h}", bufs=2) for h in range(2)]
                degps = psdeg_pool.tile([16, NBLK], f32, tag="deg",
                                        name=f"degps{sb}")
                x1ps_s[sb] = x1ps
                degps_s[sb] = degps
                for g in range(PC // GRP):
                    mt = mask_pool.tile([128, GRP, 2, NBLK], f8, tag="mt",
                                        name=f"mt{sb}_{g}", bufs=4)
                    if sb == 0 and g == 0:
                        # split the first group so the first matmuls can
                        # start after ~1/8 of it lands, with the first xq
                        # piece right behind
                        nc.sync.dma_start(mt[:, 0:1], mq[sb, g, :, 0:1])
                        xq_piece(0, 1)
                        nc.sync.dma_start(mt[:, 1:GRP], mq[sb, g, :, 1:GRP])
                        xq_piece(1, 8)
                    else:
                        nc.sync.dma_start(mt[:], mq[sb, g])
                        if sb == 0:
                            xq_piece(g * GRP, (g + 1) * GRP)
                    if after_group_dma is not None and g in after_group_dma:
                        after_group_dma[g]()
                    for u in range(GRP):
                        pc = g * GRP + u
                        st = pc == 0
                        sp = pc == PC - 1
                        for h in range(2):
                            nc.tensor.matmul(
                                x1ps[h][:], xq_sb[:, 2 * pc + h], mt[:, u],
                                start=st, stop=sp, perf_mode=DR,
                            )
                        nc.tensor.matmul(degps[:], ones_sb[:], mt[:, u],
                                         start=st, stop=sp, perf_mode=DR)
                        if interleave is not None:
                            interleave(pc)

            def emit_sweep_tail(sb):
                # deg + msumT out of PSUM (DVE); fine-sliced so stage-2 can
                # start per node tile. msumT cast to fp8 pairs (values ~±64,
                # well inside e4m3; the x1 term is ~2% of the output).
                nc.vector.tensor_copy(
                    deg_sb[:, sb * NBLK:(sb + 1) * NBLK], degps_s[sb][:]
                )
                # msumT reaches ~±300 > e4m3 max 240: store msumT/4 (the
                # host bakes the compensating 4x into w8)
                for h in range(2):
                    for jl in range(JSW):
                        j = sb * JSW + jl
                        nc.vector.tensor_scalar_mul(
                            x1t8_sb[:, h, j * 128:(j + 1) * 128],
                            x1ps_s[sb][h][:, jl * 128:(jl + 1) * 128],
                            0.25,
                        )

            def emit_transposes(sb):
                pt = ps2_pool.tile([128, 16 * JSW], f32, tag="po",
                                   name=f"psumt{sb}", bufs=3)
                psumt_s[sb] = pt
                for jl in range(JSW):
                    j = sb * JSW + jl
                    nc.tensor.transpose(
                        pt[:, jl * 16:(jl + 1) * 16],
                        deg_sb[:, j * 128:(j + 1) * 128],
                        identity[0:16, 0:16],
                    )

            def emit_recip(sb):
                nc.vector.reciprocal(
                    rec_sb[:, sb * 16 * JSW:(sb + 1) * 16 * JSW],
                    psumt_s[sb][:],
                )

            ot_tiles = {}

            def emit_s2(j, k):
                jc = slice(j * 128, (j + 1) * 128)
                po = ps2_pool.tile([128, 2 * O], f32, tag="po",
                                   name=f"po{j}_{k}", bufs=3)
                # x1 part: one fp8 DoubleRow matmul over both feature halves
                nc.tensor.matmul(po[:, 0:O], x1t8_sb[:, :, jc], w8_sb[:, k],
                                 start=True, stop=True, perf_mode=DR)
                # x part: two bf16 matmuls
                for fc in (2, 3):
                    nc.tensor.matmul(po[:, O:2 * O], xt_sb[:, fc - 2, jc],
                                     w_sb[:, k, fc],
                                     start=(fc == 2), stop=(fc == 3))
                # epilogue: scale the x1 part by rec (PSUM read) on ACT and
                # GpSimd alternately, DVE adds the x part (the other PSUM
                # read) and casts to bf16
                t1 = work_pool.tile([128, O], f32, tag="t1", bufs=3)
                nc.scalar.activation(t1[:], po[:, 0:O], Copy,
                                     scale=rec_sb[:, j * 16:j * 16 + 1])
                if k == 0:
                    ot_tiles[j] = work_pool.tile([128, 3, O], bf16, tag="ot",
                                                 name=f"ot{j}", bufs=2)
                ot = ot_tiles[j]
                nc.vector.tensor_add(ot[:, k], po[:, O:2 * O], t1[:])
                if with_bias:
                    nc.vector.tensor_add(ot[:, k], ot[:, k], bias_sb[:])
                # write each k-slice as soon as its epilogue lands
                nc.gpsimd.dma_start(out[jc, k], ot[:, k])

            # ---- emission plan
            emit_sweep(0)
            emit_sweep_tail(0)

            s2q = [(j, k) for j in range(JSW) for k in range(3)]

            def il(pc):
                if pc == 0:
                    emit_transposes(0)
                elif pc == 1:
                    emit_recip(0)
                elif pc >= 10 and pc % 2 == 0 and len(s2q) > 3:
                    # pc >= 10 keeps these after s2_inputs' emission at
                    # group 1 (their matmuls must be emitted after the
                    # writes of the tiles they read); 3 iters are held
                    # back to keep the PE busy while the DVE drains
                    # sweep-1's psum copies
                    emit_s2(*s2q.pop(0))

            def s2_inputs():
                # stage-2 inputs slot into the sync ring right after
                # sweep-1's second mask group: they land just before the
                # first interleaved stage-2 matmul needs them
                nc.sync.dma_start(xt_sb[:], xt[:])
                nc.sync.dma_start(w_sb[:], w[:])
                nc.sync.dma_start(w8_sb[:], w8[:])
                if with_bias:
                    nc.sync.dma_start(bias_sb[:], biasr[:])

            emit_sweep(1, interleave=il, after_group_dma={1: s2_inputs})
            while s2q:
                emit_s2(*s2q.pop(0))
            emit_sweep_tail(1)
            emit_transposes(1)
            emit_recip(1)
            for j in range(JSW, JT):
                for k in range(3):
                    emit_s2(j, k)

    return nc


_cached = {}


def _get_bass(with_bias: bool):
    if with_bias not in _cached:
        _cached[with_bias] = build_bass(with_bias)
    return _cached[with_bias]


def _host_prep(x, adj, weight, bias):
    import ml_dtypes

    f8 = ml_dtypes.float8_e4m3
    x = np.asarray(x, dtype=np.float32)
    adj = np.asarray(adj)
    weight = np.asarray(weight, dtype=np.float32)
    bias = np.asarray(bias, dtype=np.float32)

    with_bias = bool(np.any(bias))

    x8 = x.astype(f8)
    # xq[p, pc*2+h, i, f] = x[pc*256+i*128+p, h*128+f]
    xq = np.ascontiguousarray(
        x8.reshape(PC, 2, 128, 2, 128).transpose(2, 0, 3, 1, 4)
    ).reshape(128, PC * 2, 2, 128)
    xbf = x.astype(ml_dtypes.bfloat16)
    wbf = weight.astype(ml_dtypes.bfloat16)
    # w[p, k, fc, o] = W[k, fc*128+p, o]
    w_t = np.ascontiguousarray(
        wbf.reshape(3, 4, 128, O).transpose(2, 0, 1, 3)
    )
    # w8[p, k, i, o] = 4*W[k, i*128+p, o] for the x1 block (rows 0:256), fp8;
    # the 4x compensates the device storing msumT/4 (e4m3 range)
    w8_t = np.ascontiguousarray(
        (4.0 * weight[:, :2 * 128]).reshape(3, 2, 128, O).transpose(2, 0, 1, 3)
    ).astype(f8)
    bias_r = np.broadcast_to(bias, (128, O)).copy() if with_bias else None

    GRP = 8
    mask = adj == 1
    maskT = mask.T  # [m, node] view
    in_maps = []
    for c in range(NCORES):
        rows = slice(c * NB, (c + 1) * NB)
        blk = np.ascontiguousarray(maskT[:, rows])  # [8192 m, 1024 n] bool
        # mq[sb, g, p, u, i, n] = mask[node sb*512+n, m=(g*8+u)*256+i*128+p]
        mqc = np.ascontiguousarray(
            blk.reshape(PC // GRP, GRP, 2, 128, NSW, NBLK)
            .transpose(4, 0, 3, 1, 2, 5)
        ).astype(f8)
        # xt[p, fc, n] = x[core node n, fc*128+p]
        xt_c = np.ascontiguousarray(
            xbf[rows].reshape(NB, 2, 128).transpose(2, 1, 0)
        )
        m = {"mq": mqc, "xq": xq, "xt": xt_c, "w": w_t, "w8": w8_t}
        if with_bias:
            m["biasr"] = bias_r
        in_maps.append(m)
    return in_maps, with_bias


def run(x, adj, weight, bias, trace=False, trace_kwargs=None):
    """Shard, run on 8 cores, gather. Returns (out_full, BassKernelResults)."""
    from concourse.bass_utils import run_bass_kernel_spmd

    in_maps, with_bias = _host_prep(x, adj, weight, bias)
    nc = _get_bass(with_bias)
    res = run_bass_kernel_spmd(
        nc, in_maps, list(range(NCORES)), trace=trace, **(trace_kwargs or {})
    )
    out_full = np.empty((3, N, O), dtype=np.float32)
    for c in range(NCORES):
        out_full[:, c * NB:(c + 1) * NB, :] = (
            res.results[c]["out"].transpose(1, 0, 2).astype(np.float32)
        )
    return out_full, res


def kernel(g, x, adj, weight, bias):
    out, _ = run(x, adj, weight, bias)
    return out


# revision 42
# speedup vs baseline: 1.0168x; 1.0168x over previous
"""Trainium2 Bass kernel for CoreSageLayer (GNN mean-aggregate + 3-way linear).

Computation (reference):
    mask = (adj == 1)                      # [N, N] 0/1
    deg  = mask.sum(axis=1)                # [N]
    x1   = (mask @ x) / deg[:, None]       # [N, F]
    out[k] = concat([x1, x], 1) @ W[k] + bias   # [3, N, O]

Distribution: row-shard adj / x1 / out over nodes across 8 cores; replicate
x and weights; no collectives (rows independent).

Device schedule per core (NB=1024 nodes), fp8 DoubleRow stage 1:
  Nodes are processed in two sweeps of 512 (one PSUM bank of output each).
  Per sweep, the 8192-deep contraction streams as 32 mask half-slabs
  [128, 2, 512] (fp8, 0/1 exact); stationary = x pair-chunks [128, 2, 128]
  (fp8) giving msumT = (mask @ x)^T accumulated directly in [feature, node]
  orientation (no transposes), plus one ones-stationary matmul per slab for
  deg. Stage 2 for sweep-0 nodes is interleaved into sweep 1's PE stream;
  epilogue out[k] = (msumT_j @ W1k)*rec + xT_j @ W2k splits across the ACT
  engine (per-partition rec scale out of PSUM) and the DVE (add + bf16 cast).
  Output is written bf16 node-major [NB, 3, O] (one DMA per node tile) and
  upcast/transposed on host.
"""

import sys

sys.path.insert(0, "/opt/trn_rl_repo")

import numpy as np

N = 8192
F = 256
O = 256
NCORES = 8
NB = N // NCORES          # nodes per core (1024)
PC = N // 256             # contraction pair-chunks of 256 rows (32)
JT = NB // 128            # stage-2 node tiles per core (8)
NBLK = 512                # nodes per sweep (one PSUM bank)
NSW = NB // NBLK          # sweeps (2)
JSW = NBLK // 128         # stage-2 node tiles per sweep (4)


def _patch_tile_drain():
    """This container's walrus allows only one sync-wait per CTRL instruction;
    split the Tile kernel-tail drain's waits onto single-wait no-fuse NoOps."""
    import concourse.tile as tile
    from concourse import mybir
    from concourse.tile import ScopedClock

    if getattr(tile.TileContext, "_drain_split_patched", False):
        return

    def _drain_and_barrier(self, tick_clock, wait_clock):
        nc = self.nc
        drain_inst = nc.sync.drain()
        wait_clock.add_sem_waits(
            drain_inst.ins, ScopedClock({None: tick_clock.global_clock})
        )
        si = drain_inst.ins.sync_info
        if si is not None and len(si.on_wait) > 1:
            waits = list(si.on_wait)
            drain_inst.ins.sync_info = mybir.SyncInfo(
                on_wait=[waits[0]], on_update=list(si.on_update)
            )
            for w in waits[1:]:
                nop = nc.sync.nop(nofuse=True, hint="split_wait")
                nop.ins.sync_info = mybir.SyncInfo(on_wait=[w], on_update=[])
        nc.all_engine_barrier()
        assert self.sems is not None
        popped = nc._tile_sem_poison_stack.pop()
        assert popped is self._sem_poison
        nc.clear_and_free_semaphores(list(self.sems.allocated().values()))
        nc.all_engine_barrier()

    tile.TileContext._drain_and_barrier = _drain_and_barrier
    tile.TileContext._drain_split_patched = True

    # Same walrus limitation, general case: any instruction that Tile gave
    # >1 sem-wait (e.g. a DMA with both RAW and WAR deps) fails codegen.
    # Split surplus waits onto fresh single-wait NoOps emitted just before
    # the instruction on the same engine, at the serialized-BIR level.
    import concourse.bass as bass
    import orjson

    _orig_to_json_bytes = bass.Bass.to_json_bytes

    def _to_json_bytes_split(self):
        m = orjson.loads(_orig_to_json_bytes(self))
        ctr = 0
        for fn in m.get("functions", []):
            for bb in fn.get("blocks", []):
                insts = bb.get("instructions", [])
                new = []
                for inst in insts:
                    si = inst.get("sync_info")
                    waits = (si or {}).get("on_wait") or []
                    if len(waits) > 1:
                        for w in waits[:-1]:
                            ctr += 1
                            new.append({
                                "name": f"SWNOP-{ctr}",
                                "opcode": "NoOp",
                                "engine": inst["engine"],
                                "ins": [],
                                "outs": [],
                                "sync_info": {"on_wait": [w], "on_update": []},
                            })
                        si["on_wait"] = [waits[-1]]
                    new.append(inst)
                bb["instructions"] = new
        return orjson.dumps(m)

    bass.Bass.to_json_bytes = _to_json_bytes_split


def build_bass(with_bias: bool):
    import concourse.bass as bass
    import concourse.tile as tile
    from concourse import mybir
    from concourse.masks import make_identity

    _patch_tile_drain()

    f8 = mybir.dt.float8e4
    f32 = mybir.dt.float32
    bf16 = mybir.dt.bfloat16
    DR = mybir.MatmulPerfMode.DoubleRow
    Copy = mybir.ActivationFunctionType.Copy

    GRP = 8  # pair-chunks per mask DMA (1MB transfers for full DMA-engine BW)

    nc = bass.Bass()
    # mask half-slabs, grouped for DMA:
    # [sb, g, p, u, i, n] = mask[node sb*512+n, m=(g*8+u)*256+i*128+p]
    mq = nc.dram_tensor("mq", [NSW, PC // GRP, 128, GRP, 2, NBLK], f8,
                        kind="ExternalInput")
    # x stationary pair-chunks: [p, pc*2+h, i, f] = x[pc*256+i*128+p, h*128+f]
    xq = nc.dram_tensor("xq", [128, PC * 2, 2, 128], f8, kind="ExternalInput")
    # stage-2 x^T: [p, fc, n] = x[core node n, fc*128+p]
    xt = nc.dram_tensor("xt", [128, 2, NB], bf16, kind="ExternalInput")
    # weights: [p, k, fc, o] = W[k, fc*128+p, o]
    w = nc.dram_tensor("w", [128, 3, 4, O], bf16, kind="ExternalInput")
    # x1-block weights as fp8 pairs for the stage-2 DoubleRow matmul:
    # [p, k, i, o] = W[k, i*128+p, o]
    w8 = nc.dram_tensor("w8", [128, 3, 2, O], f8, kind="ExternalInput")
    if with_bias:
        biasr = nc.dram_tensor("biasr", [128, O], f32, kind="ExternalInput")
    # node-major output: [n, k, o] (host transposes back to [3, NB, O])
    out = nc.dram_tensor("out", [NB, 3, O], bf16, kind="ExternalOutput")

    with tile.TileContext(nc) as tc:
        # two pools only: each pool exit costs a multi-barrier cleanup round
        # in the kernel tail (~1µs apiece)
        with (
            tc.tile_pool(name="sb", bufs=1) as const_pool,
            tc.tile_pool(name="ps", bufs=1, space="PSUM") as ps_pool,
        ):
            mask_pool = work_pool = const_pool
            psx1_pool = psdeg_pool = ps2_pool = ps_pool
            xt_sb = const_pool.tile([128, 2, NB], bf16)
            w_sb = const_pool.tile([128, 3, 4, O], bf16)
            w8_sb = const_pool.tile([128, 3, 2, O], f8)
            if with_bias:
                bias_sb = const_pool.tile([128, O], f32)

            xq_sb = const_pool.tile([128, PC * 2, 2, 128], f8)
            ones_sb = const_pool.tile([128, 2, 16], f8)
            nc.vector.memset(ones_sb[:], 0.0)
            nc.vector.memset(ones_sb[:, :, 0:1], 1.0)
            # warm the ACT table during the initial DMA wait: the first
            # ACTIVATE otherwise pays a 1.3µs ACT_TABLE_LOAD mid-kernel
            actwarm = const_pool.tile([128, 1], f32)
            nc.vector.memset(actwarm[:], 1.0)
            nc.scalar.activation(actwarm[:], actwarm[:], Copy, scale=1.0)
            identity = const_pool.tile([128, 128], f32)
            make_identity(nc, identity)
            # msumT as fp8 feature-pairs: [p, i, n] = msumT[i*128+p, n]
            x1t8_sb = const_pool.tile([128, 2, NB], f8)
            deg_sb = const_pool.tile([16, NB], f32)
            rec_sb = const_pool.tile([128, 16 * JT], f32)

            # xq pieces ride the same sync queue as the mask groups, in
            # consumption order (per-queue FIFO = DMA priority; a second
            # queue would steal engines from the critical path)
            def xq_piece(lo, hi):
                nc.sync.dma_start(xq_sb[:, 2 * lo:2 * hi],
                                  xq[:, 2 * lo:2 * hi])

            x1ps_s = {}
            degps_s = {}
            psumt_s = {}

            def emit_sweep(sb, interleave=None, after_group_dma=None):
                x1ps = [psx1_pool.tile([128, NBLK], f32, tag=f"x1_{h}",
                                       name=f"x1ps{sb}{h}", bufs=2) for h in range(2)]
                degps = psdeg_pool.tile([16, NBLK], f32, tag="deg",
                                        name=f"degps{sb}")
                x1ps_s[sb] = x1ps
                degps_s[sb] = degps
                for g in range(PC // GRP):
                    mt = mask_pool.tile([128, GRP, 2, NBLK], f8, tag="mt",
                                        name=f"mt{sb}_{g}", bufs=4)
                    if sb == 0 and g == 0:
                        # split the first group so the first matmuls can
                        # start after ~1/8 of it lands, with the first xq
                        # piece right behind
                        nc.sync.dma_start(mt[:, 0:2], mq[sb, g, :, 0:2])
                        xq_piece(0, 2)
                        nc.sync.dma_start(mt[:, 2:GRP], mq[sb, g, :, 2:GRP])
                        xq_piece(2, 8)
                    else:
                        nc.sync.dma_start(mt[:], mq[sb, g])
                        if sb == 0:
                            xq_piece(g * GRP, (g + 1) * GRP)
                    if after_group_dma is not None and g in after_group_dma:
                        after_group_dma[g]()
                    for u in range(GRP):
                        pc = g * GRP + u
                        st = pc == 0
                        sp = pc == PC - 1
                        for h in range(2):
                            nc.tensor.matmul(
                                x1ps[h][:], xq_sb[:, 2 * pc + h], mt[:, u],
                                start=st, stop=sp, perf_mode=DR,
                            )
                        nc.tensor.matmul(degps[:], ones_sb[:], mt[:, u],
                                         start=st, stop=sp, perf_mode=DR)
                        if interleave is not None:
                            interleave(pc)

            def emit_sweep_tail(sb):
                # deg + msumT out of PSUM (DVE); fine-sliced so stage-2 can
                # start per node tile. msumT cast to fp8 pairs (values ~±64,
                # well inside e4m3; the x1 term is ~2% of the output).
                nc.vector.tensor_copy(
                    deg_sb[:, sb * NBLK:(sb + 1) * NBLK], degps_s[sb][:]
                )
                # msumT reaches ~±300 > e4m3 max 240: store msumT/4 (the
                # host bakes the compensating 4x into w8)
                for h in range(2):
                    for jl in range(JSW):
                        j = sb * JSW + jl
                        nc.vector.tensor_scalar_mul(
                            x1t8_sb[:, h, j * 128:(j + 1) * 128],
                            x1ps_s[sb][h][:, jl * 128:(jl + 1) * 128],
                            0.25,
                        )

            def emit_transposes(sb):
                pt = ps2_pool.tile([128, 16 * JSW], f32, tag="po",
                                   name=f"psumt{sb}", bufs=3)
                psumt_s[sb] = pt
                for jl in range(JSW):
                    j = sb * JSW + jl
                    nc.tensor.transpose(
                        pt[:, jl * 16:(jl + 1) * 16],
                        deg_sb[:, j * 128:(j + 1) * 128],
                        identity[0:16, 0:16],
                    )

            def emit_recip(sb):
                nc.vector.reciprocal(
                    rec_sb[:, sb * 16 * JSW:(sb + 1) * 16 * JSW],
                    psumt_s[sb][:],
                )

            ot_tiles = {}

            def emit_s2(j, k):
                jc = slice(j * 128, (j + 1) * 128)
                po = ps2_pool.tile([128, 2 * O], f32, tag="po",
                                   name=f"po{j}_{k}", bufs=3)
                # x1 part: one fp8 DoubleRow matmul over both feature halves
                nc.tensor.matmul(po[:, 0:O], x1t8_sb[:, :, jc], w8_sb[:, k],
                                 start=True, stop=True, perf_mode=DR)
                # x part: two bf16 matmuls
                for fc in (2, 3):
                    nc.tensor.matmul(po[:, O:2 * O], xt_sb[:, fc - 2, jc],
                                     w_sb[:, k, fc],
                                     start=(fc == 2), stop=(fc == 3))
                # epilogue: scale the x1 part by rec (PSUM read) on ACT and
                # GpSimd alternately, DVE adds the x part (the other PSUM
                # read) and casts to bf16
                t1 = work_pool.tile([128, O], f32, tag="t1", bufs=3)
                nc.scalar.activation(t1[:], po[:, 0:O], Copy,
                                     scale=rec_sb[:, j * 16:j * 16 + 1])
                if k == 0:
                    ot_tiles[j] = work_pool.tile([128, 3, O], bf16, tag="ot",
                                                 name=f"ot{j}", bufs=2)
                ot = ot_tiles[j]
                nc.vector.tensor_add(ot[:, k], po[:, O:2 * O], t1[:])
                if with_bias:
                    nc.vector.tensor_add(ot[:, k], ot[:, k], bias_sb[:])
                # write each k-slice as soon as its epilogue lands
                nc.gpsimd.dma_start(out[jc, k], ot[:, k])

            # ---- emission plan
            emit_sweep(0)
            emit_sweep_tail(0)

            s2q = [(j, k) for j in range(JSW) for k in range(3)]

            def il(pc):
                if pc == 0:
                    emit_transposes(0)
                elif pc == 1:
                    emit_recip(0)
                elif pc >= 10 and pc % 2 == 0 and len(s2q) > 3:
                    # pc >= 10 keeps these after s2_inputs' emission at
                    # group 1 (their matmuls must be emitted after the
                    # writes of the tiles they read); 3 iters are held
                    # back to keep the PE busy while the DVE drains
                    # sweep-1's psum copies
                    emit_s2(*s2q.pop(0))

            def s2_inputs():
                # stage-2 inputs slot into the sync ring right after
                # sweep-1's second mask group: they land just before the
                # first interleaved stage-2 matmul needs them
                nc.sync.dma_start(xt_sb[:], xt[:])
                nc.sync.dma_start(w_sb[:], w[:])
                nc.sync.dma_start(w8_sb[:], w8[:])
                if with_bias:
                    nc.sync.dma_start(bias_sb[:], biasr[:])

            emit_sweep(1, interleave=il, after_group_dma={1: s2_inputs})
            # sweep-1 psum copies go on the DVE queue FIRST, so they run
            # while the PE chews the held-back stage-2 iters
            emit_sweep_tail(1)
            while s2q:
                emit_s2(*s2q.pop(0))
            emit_transposes(1)
            emit_recip(1)
            for j in range(JSW, JT):
                for k in range(3):
                    emit_s2(j, k)

    return nc


_cached = {}


def _get_bass(with_bias: bool):
    if with_bias not in _cached:
        _cached[with_bias] = build_bass(with_bias)
    return _cached[with_bias]


def _host_prep(x, adj, weight, bias):
    import ml_dtypes

    f8 = ml_dtypes.float8_e4m3
    x = np.asarray(x, dtype=np.float32)
    adj = np.asarray(adj)
    weight = np.asarray(weight, dtype=np.float32)
    bias = np.asarray(bias, dtype=np.float32)

    with_bias = bool(np.any(bias))

    x8 = x.astype(f8)
    # xq[p, pc*2+h, i, f] = x[pc*256+i*128+p, h*128+f]
    xq = np.ascontiguousarray(
        x8.reshape(PC, 2, 128, 2, 128).transpose(2, 0, 3, 1, 4)
    ).reshape(128, PC * 2, 2, 128)
    xbf = x.astype(ml_dtypes.bfloat16)
    wbf = weight.astype(ml_dtypes.bfloat16)
    # w[p, k, fc, o] = W[k, fc*128+p, o]
    w_t = np.ascontiguousarray(
        wbf.reshape(3, 4, 128, O).transpose(2, 0, 1, 3)
    )
    # w8[p, k, i, o] = 4*W[k, i*128+p, o] for the x1 block (rows 0:256), fp8;
    # the 4x compensates the device storing msumT/4 (e4m3 range)
    w8_t = np.ascontiguousarray(
        (4.0 * weight[:, :2 * 128]).reshape(3, 2, 128, O).transpose(2, 0, 1, 3)
    ).astype(f8)
    bias_r = np.broadcast_to(bias, (128, O)).copy() if with_bias else None

    GRP = 8
    mask = adj == 1
    maskT = mask.T  # [m, node] view
    in_maps = []
    for c in range(NCORES):
        rows = slice(c * NB, (c + 1) * NB)
        blk = np.ascontiguousarray(maskT[:, rows])  # [8192 m, 1024 n] bool
        # mq[sb, g, p, u, i, n] = mask[node sb*512+n, m=(g*8+u)*256+i*128+p]
        mqc = np.ascontiguousarray(
            blk.reshape(PC // GRP, GRP, 2, 128, NSW, NBLK)
            .transpose(4, 0, 3, 1, 2, 5)
        ).astype(f8)
        # xt[p, fc, n] = x[core node n, fc*128+p]
        xt_c = np.ascontiguousarray(
            xbf[rows].reshape(NB, 2, 128).transpose(2, 1, 0)
        )
        m = {"mq": mqc, "xq": xq, "xt": xt_c, "w": w_t, "w8": w8_t}
        if with_bias:
            m["biasr"] = bias_r
        in_maps.append(m)
    return in_maps, with_bias


def run(x, adj, weight, bias, trace=False, trace_kwargs=None):
    """Shard, run on 8 cores, gather. Returns (out_full, BassKernelResults)."""
    from concourse.bass_utils import run_bass_kernel_spmd

    in_maps, with_bias = _host_prep(x, adj, weight, bias)
    nc = _get_bass(with_bias)
    res = run_bass_kernel_spmd(
        nc, in_maps, list(range(NCORES)), trace=trace, **(trace_kwargs or {})
    )
    out_full = np.empty((3, N, O), dtype=np.float32)
    for c in range(NCORES):
        out_full[:, c * NB:(c + 1) * NB, :] = (
            res.results[c]["out"].transpose(1, 0, 2).astype(np.float32)
        )
    return out_full, res


def kernel(g, x, adj, weight, bias):
    out, _ = run(x, adj, weight, bias)
    return out
